# revision 3
# baseline (speedup 1.0000x reference)
"""Trainium2 Bass kernel for nn_AutoregressiveDecoder (gnn_message_passing).

reference math (N=512, D=256, H=64):
    x = z @ z.T
    M[i,r] = r < i;  colsum = (M @ adj) * M;  degs = max(colsum,1)^-0.5
    base = z @ W1[:256]          (the W1[-1] one-hot helper row is provably
                                  dead: spconv masks row i to zero before it
                                  can propagate)
    per i:  d_i = M[i] * degs[i]            (>=0, zero for r>=i)
            Y_i   = adj @ (d_i * base)       [N,H]
            s_i   = (d_i * relu(Y_i)) @ W2   [N]     (relu(d*Y)=d*relu(Y), d>=0)
            t_i   = d_i * s_i
            S[i]  = d_i * (adj @ t_i)        [N]
    out = x + 0.5*(S + S.T)

Distribution: the vmapped i axis is sharded over 8 cores in interleaved
chunks of 16 (core k gets chunks k, k+8, k+16, k+24) so the triangular
prefix bound b = 16c+16 (only nodes r < i participate) load-balances:
every core sees bounds {128,256,384,512}. adj/z/W1/W2 replicated.
Each core returns its 64 output rows (x + 0.5*S) plus its S^T column
shard; the host gather step assembles out += 0.5*S^T.
"""
import sys

sys.path.insert(0, "/opt/trn_rl_repo")

import numpy as np
import ml_dtypes

N = 512
D = 256
H = 64
NCORES = 8
NI = 16            # i per chunk
NCHUNKS = N // NI  # 32
CPC = NCHUNKS // NCORES  # 4 chunks per core
P = 128
KT = N // P        # 4 partition/K tiles
BF = ml_dtypes.bfloat16

_cache = {}


def _chunks_of_core(k):
    return [k + NCORES * g for g in range(CPC)]


def _iset_of_core(k):
    out = []
    for c in _chunks_of_core(k):
        out.extend(range(NI * c, NI * (c + 1)))
    return np.array(out, dtype=np.int64)


def _build():
    import concourse.bacc as bacc
    import concourse.mybir as mybir
    from concourse import tile

    fp32 = mybir.dt.float32
    bf16 = mybir.dt.bfloat16
    AT = mybir.AluOpType

    nc = bacc.Bacc("TRN2", target_bir_lowering=False, debug=False, num_devices=NCORES)

    adj_in = nc.dram_tensor("adjbf", [N, N], bf16, kind="ExternalInput")
    zt_in = nc.dram_tensor("zT", [D, N], fp32, kind="ExternalInput")
    w1_in = nc.dram_tensor("W1c", [D, H], fp32, kind="ExternalInput")
    w2_in = nc.dram_tensor("W2rep", [P, H], fp32, kind="ExternalInput")
    mtb_in = nc.dram_tensor("MTbf", [N, H], bf16, kind="ExternalInput")
    mtf_in = nc.dram_tensor("MTf", [N, H], fp32, kind="ExternalInput")
    ztk_in = nc.dram_tensor("zTk", [D, H], fp32, kind="ExternalInput")
    id_in = nc.dram_tensor("ident", [P, P], fp32, kind="ExternalInput")
    chunks_in = nc.dram_tensor("mychunks", [1, CPC], mybir.dt.int32, kind="ExternalInput")

    pout = nc.dram_tensor("pout", [H, N], fp32, kind="ExternalOutput")
    stout = nc.dram_tensor("stout", [N, H], fp32, kind="ExternalOutput")

    # per-core chunk bounds are data (mychunks) but the *shapes* must be static;
    # all cores share the same KT sequence {1,2,3,4} by construction, so the
    # graph is SPMD-identical: chunk g of core k has bound 128*(g+1).
    with tile.TileContext(nc) as tc:
        with (
            tc.tile_pool(name="const", bufs=1) as cpool,
            tc.tile_pool(name="work", bufs=3) as wpool,
            tc.tile_pool(name="ps", bufs=8, space="PSUM") as pspool,
        ):
            # ---- persistent loads (folded layout [P, kt, cols]) ----
            G = cpool.tile([P, KT, N], bf16, tag="G")
            nc.sync.dma_start(
                out=G[:, :, :], in_=adj_in.ap().rearrange("(kt p) c -> p kt c", p=P)
            )
            zT = cpool.tile([P, D // P, N], fp32, tag="zT")
            nc.sync.dma_start(
                out=zT[:, :, :], in_=zt_in.ap().rearrange("(kt p) c -> p kt c", p=P)
            )
            W1c = cpool.tile([P, D // P, H], fp32, tag="W1c")
            nc.sync.dma_start(
                out=W1c[:, :, :], in_=w1_in.ap().rearrange("(kt p) h -> p kt h", p=P)
            )
            W2rep = cpool.tile([P, H], fp32, tag="W2rep")
            nc.sync.dma_start(out=W2rep[:, :], in_=w2_in[:, :])
            MTb = cpool.tile([P, KT, H], bf16, tag="MTb")
            nc.sync.dma_start(
                out=MTb[:, :, :], in_=mtb_in.ap().rearrange("(kt p) i -> p kt i", p=P)
            )
            MTf = cpool.tile([P, KT, H], fp32, tag="MTf")
            nc.sync.dma_start(
                out=MTf[:, :, :], in_=mtf_in.ap().rearrange("(kt p) i -> p kt i", p=P)
            )
            zTk = cpool.tile([P, D // P, H], fp32, tag="zTk")
            nc.sync.dma_start(
                out=zTk[:, :, :], in_=ztk_in.ap().rearrange("(kt p) i -> p kt i", p=P)
            )
            ident = cpool.tile([P, P], fp32, tag="ident")
            nc.sync.dma_start(out=ident[:, :], in_=id_in[:, :])

            # ---- colsumT[r, i] = sum_{r'<i} adj[r', r] = (M @ adj).T slice ----
            # out[pb] = sum_kt adjbf[kt, pb-cols].T @ MTb[kt]
            cs_ps = []
            for pb in range(KT):
                ps = pspool.tile([P, H], fp32, tag="ps")
                cs_ps.append(ps)
                for kt in range(KT):
                    nc.tensor.matmul(
                        ps[:, :],
                        G[:, kt, pb * P : (pb + 1) * P],
                        MTb[:, kt, :],
                        start=(kt == 0),
                        stop=(kt == KT - 1),
                    )

            # ---- d = MT * (max(colsumT,1))^-1/2 ; dT2 = d*d ; dTb = bf16(d) ----
            mx = cpool.tile([P, KT, H], fp32, tag="mx")
            for pb in range(KT):
                nc.vector.tensor_scalar_max(
                    out=mx[:, pb, :], in0=cs_ps[pb][:, :], scalar1=1.0
                )
            rc = cpool.tile([P, KT, H], fp32, tag="rc")
            nc.vector.reciprocal(out=rc[:, :, :], in_=mx[:, :, :])
            sq = cpool.tile([P, KT, H], fp32, tag="sq")
            nc.scalar.activation(
                out=sq[:, :, :], in_=rc[:, :, :], func=mybir.ActivationFunctionType.Sqrt
            )
            dT = cpool.tile([P, KT, H], fp32, tag="dT")
            nc.vector.tensor_tensor(
                out=dT[:, :, :], in0=sq[:, :, :], in1=MTf[:, :, :], op=AT.mult
            )
            dT2 = cpool.tile([P, KT, H], fp32, tag="dT2")
            nc.vector.tensor_tensor(
                out=dT2[:, :, :], in0=dT[:, :, :], in1=dT[:, :, :], op=AT.mult
            )
            dTb = cpool.tile([P, KT, H], bf16, tag="dTb")
            nc.vector.tensor_copy(out=dTb[:, :, :], in_=dT[:, :, :])

            # ---- base = z @ W1c -> [N, H]; bf16 copy ----
            bb_ps = []
            for pb in range(KT):
                ps = pspool.tile([P, H], fp32, tag="ps")
                bb_ps.append(ps)
                for kt in range(D // P):
                    nc.tensor.matmul(
                        ps[:, :],
                        zT[:, kt, pb * P : (pb + 1) * P],
                        W1c[:, kt, :],
                        start=(kt == 0),
                        stop=(kt == D // P - 1),
                    )
            bbf = cpool.tile([P, KT, H], bf16, tag="bbf")
            for pb in range(KT):
                nc.vector.tensor_copy(out=bbf[:, pb, :], in_=bb_ps[pb][:, :])

            # ---- T (t columns for my 64 i's), zero rows above prefix bound ----
            Tb = cpool.tile([P, KT, H], bf16, tag="Tb")
            nc.vector.memset(Tb[:, :, :], 0.0)

            # ---- main loop over my 4 chunks ----
            for g in range(CPC):
                kts = g + 1  # prefix bound in 128-tiles: 128*(g+1)
                icol0 = g * NI  # this chunk's 16 columns within my 64 i-slots
                # V[r, (i,h)] = dTb[r, i] * bbf[r, h]   (rows < 128*kts)
                V = wpool.tile([P, kts, NI, H], bf16, tag="V")
                nc.vector.tensor_tensor(
                    out=V[:, :, :, :],
                    in0=bbf[:, 0:kts, :].unsqueeze(2).broadcast_to((P, kts, NI, H)),
                    in1=dTb[:, 0:kts, icol0 : icol0 + NI]
                    .unsqueeze(3)
                    .broadcast_to((P, kts, NI, H)),
                    op=AT.mult,
                )
                RW = wpool.tile([P, kts, NI, H], bf16, tag="RW")
                NCC = NI * H // 512  # 2 col-chunks of 512
                for pb in range(kts):
                    for cc in range(NCC):
                        yps = pspool.tile([P, 512], fp32, tag="ps")
                        for kt in range(kts):
                            nc.tensor.matmul(
                                yps[:, :],
                                G[:, kt, pb * P : (pb + 1) * P],
                                V[:, kt, :, :].rearrange("p i h -> p (i h)")[
                                    :, cc * 512 : (cc + 1) * 512
                                ],
                                start=(kt == 0),
                                stop=(kt == kts - 1),
                            )
                        # RW = relu(Y) * W2   (one DVE pass, PSUM read)
                        nio = NI // NCC
                        nc.vector.scalar_tensor_tensor(
                            out=RW[:, pb, cc * nio : (cc + 1) * nio, :],
                            in0=yps[:, :].rearrange("p (i h) -> p i h", h=H),
                            scalar=0.0,
                            in1=W2rep[:, :].unsqueeze(1).broadcast_to((P, nio, H)),
                            op0=AT.max,
                            op1=AT.mult,
                        )
                # s_pre[r, i] = sum_h RW ; t = s_pre * d^2
                spre = wpool.tile([P, kts, NI], fp32, tag="spre")
                nc.vector.tensor_reduce(
                    out=spre[:, :, :],
                    in_=RW[:, :, :, :],
                    axis=mybir.AxisListType.X,
                    op=AT.add,
                )
                nc.vector.tensor_tensor(
                    out=Tb[:, 0:kts, icol0 : icol0 + NI],
                    in0=spre[:, :, :],
                    in1=dT2[:, 0:kts, icol0 : icol0 + NI],
                    op=AT.mult,
                )

            # ---- O = adj @ T ; ST = d * O  (S^T columns for my i's) ----
            STf = cpool.tile([P, KT, H], fp32, tag="STf")
            for pb in range(KT):
                ops = pspool.tile([P, H], fp32, tag="ps")
                for kt in range(KT):
                    nc.tensor.matmul(
                        ops[:, :],
                        G[:, kt, pb * P : (pb + 1) * P],
                        Tb[:, kt, :],
                        start=(kt == 0),
                        stop=(kt == KT - 1),
                    )
                nc.vector.tensor_tensor(
                    out=STf[:, pb, :], in0=ops[:, :], in1=dT[:, pb, :], op=AT.mult
                )
            nc.sync.dma_start(
                out=stout.ap().rearrange("(pb p) i -> p pb i", p=P), in_=STf[:, :, :]
            )

            # ---- S rows for my i's: transpose ST blocks -> [64, 512] ----
            strans = pspool.tile([H, N], fp32, tag="ps")
            for pb in range(KT):
                nc.tensor.transpose(
                    out=strans[:, pb * P : (pb + 1) * P],
                    in_=STf[:, pb, :],
                    identity=ident[:, :],
                )
            ssb = cpool.tile([H, N], fp32, tag="ssb")
            nc.scalar.activation(
                out=ssb[:, :], in_=strans[:, :], func=mybir.ActivationFunctionType.Copy
            )

            # ---- x rows: z[my i] @ z.T ----
            xps = pspool.tile([H, N], fp32, tag="ps")
            for kt in range(D // P):
                nc.tensor.matmul(
                    xps[:, :],
                    zTk[:, kt, :],
                    zT[:, kt, :],
                    start=(kt == 0),
                    stop=(kt == D // P - 1),
                )

            # ---- pout = x + 0.5 * S_rows ----
            po = cpool.tile([H, N], fp32, tag="po")
            nc.vector.scalar_tensor_tensor(
                out=po[:, :],
                in0=ssb[:, :],
                scalar=0.5,
                in1=xps[:, :],
                op0=AT.mult,
                op1=AT.add,
            )
            nc.sync.dma_start(out=pout[:, :], in_=po[:, :])
            # keep chunks_in alive as a parameter (unused on device)
            dummy = cpool.tile([1, CPC], mybir.dt.int32, tag="dummy")
            nc.sync.dma_start(out=dummy[:, :], in_=chunks_in[:, :])

    nc.compile()
    return nc


def _get_nc():
    if "nc" not in _cache:
        _cache["nc"] = _build()
    return _cache["nc"]


def _prepare_in_maps(z, adj, W1, W2):
    z = np.asarray(z, dtype=np.float32)
    adj = np.asarray(adj, dtype=np.float32)
    W1 = np.asarray(W1, dtype=np.float32)
    W2 = np.asarray(W2, dtype=np.float32)

    adjbf = adj.astype(BF)  # 0/1 values: exact in bf16
    zT = np.ascontiguousarray(z.T)
    W1c = np.ascontiguousarray(W1[:D])
    W2rep = np.tile(W2.reshape(1, H), (P, 1)).astype(np.float32)
    ident = np.eye(P, dtype=np.float32)

    idx = np.arange(N)
    in_maps = []
    for k in range(NCORES):
        iset = _iset_of_core(k)
        MT = (idx[:, None] < iset[None, :]).astype(np.float32)  # [N, 64] r < i
        in_maps.append(
            {
                "adjbf": adjbf,
                "zT": zT,
                "W1c": W1c,
                "W2rep": W2rep,
                "MTbf": MT.astype(BF),
                "MTf": MT,
                "zTk": np.ascontiguousarray(zT[:, iset]),
                "ident": ident,
                "mychunks": np.array([_chunks_of_core(k)], dtype=np.int32),
            }
        )
    return in_maps


def kernel(z, adj, W1, W2):
    from concourse import bass_utils

    nc = _get_nc()
    in_maps = _prepare_in_maps(z, adj, W1, W2)
    res = bass_utils.run_bass_kernel_spmd(
        nc, in_maps, core_ids=list(range(NCORES)), trace=False
    )
    out = np.empty((N, N), dtype=np.float32)
    stf = np.empty((N, N), dtype=np.float32)
    for k in range(NCORES):
        iset = _iset_of_core(k)
        out[iset, :] = res.results[k]["pout"]
        stf[:, iset] = res.results[k]["stout"]
    # stf[r, i] = S[i, r]  =>  stf[i, c] = S[c, i]; out needs += 0.5*S[c,i] at [i,c]
    out += 0.5 * stf
    return out
